# revision 26
# baseline (speedup 1.0000x reference)
"""DeepFM forward on Trainium2, 8 NeuronCores, data-parallel over batch.

Reference computes (B=512, n=512, K=4, H=128, n_pairs=130816):
    S  = fm_w @ fm_w.T
    fm = x[:, i1] * x[:, i2] * S[i1, i2]        # [B, n_pairs]
    h2 = relu(relu(x@w1+b1)@w2+b2)
    out = sigmoid(concat([fm, h2]) @ wo + bo)

The fm @ wo[:n_pairs] contraction is the bilinear form
    t1[b] = x[b]^T Wp' x[b]  with  Wp'[i,j] = S[i,j] * Wp[i,j]
where Wp is wo[:n_pairs] scattered into the strictly-upper triangle of a
[n, n] matrix (host-side static relayout). S = fm_w fm_w^T has rank 4, so
    t1[b] = sum_t z_t[b]^T Wp z_t[b],  z_t = x * fm_w[:, t]
and only the 10 upper-triangular 128x128 blocks of Wp are shipped.

All matmul operands are fp8e4 (TRN e4m3, +-240) with power-of-2 scales
chosen so every tensor sits mid-range; the final sigmoid applies the
inverse scale. This halves HBM traffic vs bf16 (the DMA is the dominant
cost) with ~7e-4 relative error (threshold 2e-2).

Everything lives in the [batch, t] free layout so the final fold over t
is one Vector tensor_reduce. Per-core program (BC=64 batch cols):
    z_k[:, b, t] = xT_k * fmw16[k][:, t]        (DVE/Pool, fp8 out)
    VT_j = sum_{k<=j} Wp[k,j]^T @ z_k           (PE fp8 DoubleRow: adjacent
                                                 k-pairs in one instruction)
    Q_j  = VT_j * z_j                           (DVE, bf16)
    t_ps[1, b, t] += ones^T @ Q_j               (PE)
    h1 = relu8(w1^T xT + 16 b1); h2 = relu16(w2^T h1 + 32 b2)  (PE+ACT)
    t_ps[1, b, 0] += (2048 woh)^T @ h2          (PE, strided psum slot)
    t = reduce_t(t_ps); out = sigmoid(t/65536 + bo)            (DVE+ACT)

DMA plan (fixed ~700ns per dma_start on the issuing queue; transfers
share the SDMA rings): sync carries critA (x, fm_w, biases, wp00) then
critB (wp j2/j3 blocks); scalar carries critC (w1, w2, wp j1) in
parallel. PE is HAM-warmed with dummy fp8 matmuls during the DMA wait,
and a few post-output dummy ops keep the clock up into the teardown.
"""

import os
import sys

import numpy as np

for _p in ("/opt/trn_rl_repo", "/root/.axon_site/_ro/trn_rl_repo"):
    if os.path.isdir(_p) and _p not in sys.path:
        sys.path.insert(0, _p)

import ml_dtypes

import concourse.bass as bass
import concourse.tile as tile
from concourse import bacc, mybir
from concourse.bass_utils import run_bass_kernel_spmd

F32 = mybir.dt.float32
BF16 = mybir.dt.bfloat16
FP8 = mybir.dt.float8e4
AF = mybir.ActivationFunctionType
ALU = mybir.AluOpType

N = 512          # n_feat
KFM = 4          # fm embedding dim
H = 128          # mlp hidden
NP = N * (N - 1) // 2
B = 512
N_CORES = 8
BC = B // N_CORES  # 64 batch rows per core
NCH = N // 128     # 4 feature chunks

# fp8 scales (powers of two)
S_FMW = 16.0
S_WP = 256.0
S_W1 = 16.0
S_W2 = 2.0
S_T = S_FMW * S_FMW * S_WP          # 65536 on both t contributions
S_WOH = S_T / (S_W1 * S_W2)         # 2048

N_WARM = int(os.environ.get("DFM_N_WARM", "8"))
WARM_COLS = int(os.environ.get("DFM_WARM_COLS", "512"))
RELU_ENG = os.environ.get("DFM_RELU_ENG", "scalar")
Z3_ENG = os.environ.get("DFM_Z3_ENG", "gpsimd")
HAM_TAIL = int(os.environ.get("DFM_HAM_TAIL", "2"))
B_ENG = os.environ.get("DFM_B_ENG", "sync")      # sync | scalar | gpsimd
DEEP_SLOT = os.environ.get("DFM_DEEP_SLOT", "1") == "1"
PRIME_DMA = os.environ.get("DFM_PRIME_DMA", "0") == "1"
HOST_SIG = os.environ.get("DFM_HOST_SIG", "0") == "1"
DOUBLE_ROW = os.environ.get("DFM_DR", "1") == "1"
# gpsimd cannot read PSUM (walrus rejects it) — Q muls stay on Vector
Q2_ENG = os.environ.get("DFM_Q2_ENG", "vector")

# critA image (fp8 bytes): [ xT (4*64) | f32 pack (19*4) | woh bf16 | ones bf16 | wp(0,0) ]
XT_OFF = 0
F32_OFF = NCH * BC               # 256
F32_COLS = NCH * KFM + 3         # fmw16 | b1*16 | b2*32 | bo  -> 19
BF_OFF = F32_OFF + F32_COLS * 4  # 332
WPA_OFF = BF_OFF + 4             # 336
A_COLS = WPA_OFF + 128           # 464
PK_B1 = NCH * KFM                # 16
PK_B2 = PK_B1 + 1
PK_BO = PK_B2 + 1

# critC: [ w1*16 (4*128) | w2*2 (128) | wp(0,1) | wp(1,1) ]
C_W2 = NCH * H                   # 512
C_WP0 = C_W2 + H                 # 640
C_COLS = C_WP0 + 2 * 128         # 896

# critB: wp blocks (0,2),(1,2),(2,2),(0,3),(1,3),(2,3),(3,3)
B_BLOCKS = [(0, 2), (1, 2), (2, 2), (0, 3), (1, 3), (2, 3), (3, 3)]
B_COLS = len(B_BLOCKS) * 128     # 896

_IU1, _IU2 = np.triu_indices(N, k=1)

_program_cache = None


def _build_program():
    global _program_cache
    if _program_cache is not None:
        return _program_cache

    nc = bacc.Bacc(
        "TRN2", target_bir_lowering=False, debug=False, num_devices=N_CORES
    )
    critA_d = nc.declare_dram_parameter("critA", [128, A_COLS], FP8, isOutput=False)
    critB_d = nc.declare_dram_parameter("critB", [128, B_COLS], FP8, isOutput=False)
    critC_d = nc.declare_dram_parameter("critC", [128, C_COLS], FP8, isOutput=False)
    out_d = nc.declare_dram_parameter("out", [1, BC], F32, isOutput=True)

    relu_eng_is_scalar = RELU_ENG == "scalar"

    with tile.TileContext(nc) as tc:
        with (
            tc.tile_pool(name="const", bufs=1) as cpool,
            tc.tile_pool(name="work", bufs=1) as wpool,
            tc.tile_pool(name="ps_v", bufs=1, space=bass.MemorySpace.PSUM) as vpool,
            tc.tile_pool(name="ps_h", bufs=1, space=bass.MemorySpace.PSUM) as hpool,
            tc.tile_pool(name="ps_t", bufs=1, space=bass.MemorySpace.PSUM) as tpool,
        ):
            # ---- input DMAs: sync gets A then B, scalar gets C ----
            if PRIME_DMA:
                prime_sb = cpool.tile([128, 8], FP8)
                nc.gpsimd.dma_start(prime_sb[:], critB_d[:, 0:8])
            critA_sb = cpool.tile([128, A_COLS], FP8)
            nc.sync.dma_start(critA_sb[:], critA_d[:, :])
            critC_sb = cpool.tile([128, C_COLS], FP8)
            nc.scalar.dma_start(critC_sb[:], critC_d[:, :])
            critB_sb = cpool.tile([128, B_COLS], FP8)
            b_eng = {"sync": nc.sync, "scalar": nc.scalar, "gpsimd": nc.gpsimd}[B_ENG]
            b_eng.dma_start(critB_sb[:], critB_d[:, :])

            # ---- views into critA ----
            f32v = critA_sb[:, F32_OFF:BF_OFF].bitcast(F32)       # [128, 19]
            bf16v = critA_sb[:, BF_OFF:WPA_OFF].bitcast(BF16)     # [128, 2]
            woh_ap = bf16v[:, 0:1]
            ones_ap = bf16v[:, 1:2]
            b1_ap = f32v[:, PK_B1 : PK_B1 + 1]
            b2_ap = f32v[:, PK_B2 : PK_B2 + 1]
            bo_ap = f32v[0:1, PK_BO : PK_BO + 1]

            def xt(k):
                return critA_sb[:, XT_OFF + k * BC : XT_OFF + (k + 1) * BC]

            def fmw(k):
                return f32v[:, k * KFM : (k + 1) * KFM]

            wpA = critA_sb[:, WPA_OFF : WPA_OFF + 128]

            def w1c(k):
                return critC_sb[:, k * H : (k + 1) * H]

            w2_ap = critC_sb[:, C_W2 : C_W2 + H]
            wp_blk = {(0, 0): wpA}
            wp_blk[(0, 1)] = critC_sb[:, C_WP0 : C_WP0 + 128]
            wp_blk[(1, 1)] = critC_sb[:, C_WP0 + 128 : C_WP0 + 256]
            for i, kj in enumerate(B_BLOCKS):
                wp_blk[kj] = critB_sb[:, i * 128 : (i + 1) * 128]

            # ---- warm tiles (GpSimd memsets; Vector stays free) ----
            dum_lhs = cpool.tile([128, 128], FP8)
            nc.gpsimd.memset(dum_lhs[:], 0.0)
            dum_rhs = cpool.tile([128, WARM_COLS], FP8)
            nc.gpsimd.memset(dum_rhs[:], 0.0)
            warm_in = cpool.tile([1, 1], F32)
            nc.gpsimd.memset(warm_in[:], 0.0)

            warm_cols = WARM_COLS if DEEP_SLOT else BC
            warm_ps = hpool.tile(
                [128, warm_cols], F32, tag="warm" if DEEP_SLOT else "h1"
            )
            for d in range(N_WARM):
                nc.tensor.matmul(
                    warm_ps[:], dum_lhs[:], dum_rhs[:, :warm_cols],
                    start=True, stop=True,
                )
            if not HOST_SIG:
                warm_out = cpool.tile([1, 1], F32)
                nc.scalar.activation(warm_out[:], warm_in[:], AF.Sigmoid, bias=0.0)

            # ---- z_k [128, BC, KFM] fp8: x column-scaled by fm_w (rank-4) ----
            z_all = wpool.tile([128, NCH, BC, KFM], FP8, name="z_all", tag="z")

            def z(k):
                return z_all[:, k]

            for k in range(NCH):
                eng = nc.vector if (k < 3 or Z3_ENG == "vector") else nc.gpsimd
                eng.tensor_mul(
                    z(k),
                    xt(k)[:, :, None].broadcast_to([128, BC, KFM]),
                    fmw(k)[:, None, :].broadcast_to([128, BC, KFM]),
                )

            # ---- psum tiles ----
            vt = [
                vpool.tile([128, BC, KFM], F32, name=f"vt{j}", tag=f"v{j}")
                for j in range(NCH)
            ]
            h1_ps = hpool.tile([H, BC], F32, tag="h1")
            h2_ps = hpool.tile([H, BC], F32, tag="h2")
            t_ps = tpool.tile([1, BC, KFM], F32, tag="t")
            deep_out = (
                t_ps[:, :, 0:1] if DEEP_SLOT else tpool.tile([1, BC], F32, tag="deep")[:]
            )

            q_all = wpool.tile([128, NCH, BC, KFM], BF16, name="q_all", tag="q")

            def q(j):
                return q_all[:, j]

            h1_sb = wpool.tile([H, BC], FP8, name="h1_sb")
            h2_sb = wpool.tile([H, BC], BF16, name="h2_sb")

            def relu(dst, src, bias_ap):
                if relu_eng_is_scalar:
                    nc.scalar.activation(dst, src, AF.Relu, bias=bias_ap)
                else:
                    nc.vector.tensor_scalar(
                        dst, src, bias_ap, 0.0, op0=ALU.add, op1=ALU.max
                    )

            # ---- PE stream interleaved with DVE/ACT consumers ----
            # VT j0 (needs critA only)
            nc.tensor.matmul(vt[0][:], wpA, z(0), start=True, stop=True)
            # h1 (needs critC)
            for k in range(NCH):
                nc.tensor.matmul(
                    h1_ps[:], w1c(k), xt(k), start=(k == 0), stop=(k == NCH - 1)
                )
            relu(h1_sb[:], h1_ps[:], b1_ap)
            def wp_pair(base_ap, off):
                return base_ap[:, off : off + 256].rearrange(
                    "p (two f) -> p two f", two=2
                )

            def vt_dr(j, k, pair_ap, start, stop):
                nc.tensor.matmul(
                    vt[j][:], pair_ap, z_all[:, k : k + 2],
                    start=start, stop=stop,
                    perf_mode=mybir.MatmulPerfMode.DoubleRow,
                    skip_group_check=True,
                )

            # VT j1
            if DOUBLE_ROW:
                vt_dr(1, 0, wp_pair(critC_sb, C_WP0), True, True)
            else:
                nc.tensor.matmul(
                    vt[1][:], wp_blk[(0, 1)], z(0), start=True, stop=False
                )
                nc.tensor.matmul(
                    vt[1][:], wp_blk[(1, 1)], z(1), start=False, stop=True
                )
            # Q0
            nc.vector.tensor_mul(q(0), vt[0][:], z(0))
            # h2
            nc.tensor.matmul(h2_ps[:], w2_ap, h1_sb[:], start=True, stop=True)
            relu(h2_sb[:], h2_ps[:], b2_ap)
            # VT j2
            if DOUBLE_ROW:
                vt_dr(2, 0, wp_pair(critB_sb, 0), True, False)
                nc.tensor.matmul(
                    vt[2][:], wp_blk[(2, 2)], z(2),
                    start=False, stop=True, skip_group_check=True,
                )
            else:
                for i, k in enumerate(range(3)):
                    nc.tensor.matmul(
                        vt[2][:], wp_blk[(k, 2)], z(k),
                        start=(i == 0), stop=(i == 2), skip_group_check=True,
                    )
            # Q1
            nc.vector.tensor_mul(q(1), vt[1][:], z(1))
            # t accumulation group start + deep
            nc.tensor.matmul(
                t_ps[:], ones_ap, q(0), start=True, stop=False,
                skip_group_check=True,
            )
            nc.tensor.matmul(
                deep_out, woh_ap, h2_sb[:],
                start=not DEEP_SLOT, stop=not DEEP_SLOT,
                skip_group_check=True,
            )
            # VT j3
            if DOUBLE_ROW:
                vt_dr(3, 0, wp_pair(critB_sb, 384), True, False)
                vt_dr(3, 2, wp_pair(critB_sb, 640), False, True)
            else:
                for i, k in enumerate(range(4)):
                    nc.tensor.matmul(
                        vt[3][:], wp_blk[(k, 3)], z(k),
                        start=(i == 0), stop=(i == 3), skip_group_check=True,
                    )
            # Q2 off the Vector tail (GpSimd is idle by now), t1, Q3, t2, t3
            q2_eng = nc.gpsimd if Q2_ENG == "gpsimd" else nc.vector
            q2_eng.tensor_mul(q(2), vt[2][:], z(2))
            nc.tensor.matmul(
                t_ps[:], ones_ap, q(1), start=False, stop=False,
                skip_group_check=True,
            )
            nc.vector.tensor_mul(q(3), vt[3][:], z(3))
            nc.tensor.matmul(
                t_ps[:], ones_ap, q(2), start=False, stop=False,
                skip_group_check=True,
            )
            nc.tensor.matmul(
                t_ps[:], ones_ap, q(3), start=False, stop=True,
                skip_group_check=True,
            )

            # ---- fold over t (deep already in slot 0), sigmoid, store ----
            tlog_sb = wpool.tile([1, BC], F32, name="tlog")
            nc.vector.tensor_reduce(
                tlog_sb[:], t_ps[:], axis=mybir.AxisListType.X, op=ALU.add
            )
            if not DEEP_SLOT:
                tlog2 = wpool.tile([1, BC], F32, name="tlog2")
                nc.vector.tensor_add(tlog2[:], tlog_sb[:], deep_out)
                tlog_sb = tlog2
            if HOST_SIG:
                nc.sync.dma_start(out_d[:, :], tlog_sb[:])
            else:
                out_sb = wpool.tile([1, BC], F32, name="out_sb")
                nc.scalar.activation(
                    out_sb[:], tlog_sb[:], AF.Sigmoid, bias=bo_ap, scale=1.0 / S_T
                )
                nc.sync.dma_start(out_d[:, :], out_sb[:])

            # ---- keep HAM clock high into the teardown sweep ----
            for _ in range(HAM_TAIL):
                nc.tensor.matmul(
                    warm_ps[:], dum_lhs[:], dum_rhs[:, :warm_cols],
                    start=True, stop=True,
                )

    nc.compile()
    _program_cache = nc
    return nc


def _q8(a, scale):
    return np.clip(
        np.asarray(a, np.float32) * scale, -240.0, 240.0
    ).astype(ml_dtypes.float8_e4m3fn)


def _chunk_pack(a, cols):
    """[512, cols] row-major -> [128, 4, cols] -> [128, 4*cols]."""
    return np.ascontiguousarray(
        a.reshape(NCH, 128, cols).transpose(1, 0, 2).reshape(128, NCH * cols)
    )


def _prep_inputs(x, fm_w, w1, b1, w2, b2, wo, bo):
    x = np.asarray(x, dtype=np.float32)
    fm_w = np.asarray(fm_w, dtype=np.float32)
    w1 = np.asarray(w1, dtype=np.float32)
    w2 = np.asarray(w2, dtype=np.float32)
    wo = np.asarray(wo, dtype=np.float32).reshape(NP + H)
    b1 = np.asarray(b1, dtype=np.float32).reshape(H)
    b2 = np.asarray(b2, dtype=np.float32).reshape(H)
    bo = np.asarray(bo, dtype=np.float32).reshape(1)

    fp8 = ml_dtypes.float8_e4m3fn
    bf = ml_dtypes.bfloat16

    # Pair weights scattered into the strictly-upper triangle, fp8-scaled.
    wp = np.zeros((N, N), dtype=np.float32)
    wp[_IU1, _IU2] = wo[:NP]
    wp_q = _q8(wp, S_WP)

    def blk(k, j):
        return wp_q[128 * k : 128 * (k + 1), 128 * j : 128 * (j + 1)]

    critB = np.empty((128, B_COLS), dtype=fp8)
    for i, (k, j) in enumerate(B_BLOCKS):
        critB[:, i * 128 : (i + 1) * 128] = blk(k, j)
    critB = np.ascontiguousarray(critB)

    critC = np.empty((128, C_COLS), dtype=fp8)
    critC[:, :C_W2] = _chunk_pack(_q8(w1, S_W1), H)
    critC[:, C_W2:C_WP0] = _q8(w2, S_W2)
    critC[:, C_WP0 : C_WP0 + 128] = blk(0, 1)
    critC[:, C_WP0 + 128 :] = blk(1, 1)
    critC = np.ascontiguousarray(critC)

    f32_img = np.zeros((128, F32_COLS), dtype=np.float32)
    f32_img[:, :PK_B1] = _chunk_pack(fm_w * S_FMW, KFM)
    f32_img[:, PK_B1] = b1 * S_W1
    f32_img[:, PK_B2] = b2 * S_W1 * S_W2
    f32_img[:, PK_BO] = bo[0]
    bf_img = np.zeros((128, 2), dtype=bf)
    bf_img[:, 0] = (wo[NP:] * S_WOH).astype(bf)
    bf_img[:, 1] = bf(1.0)

    xT = np.ascontiguousarray(x.T)                       # [N, B] f32

    in_maps = []
    for c in range(N_CORES):
        critA = np.empty((128, A_COLS), dtype=fp8)
        critA[:, XT_OFF:F32_OFF] = _chunk_pack(
            _q8(xT[:, c * BC : (c + 1) * BC], 1.0), BC
        )
        critA[:, F32_OFF:BF_OFF] = f32_img.view(fp8)
        critA[:, BF_OFF:WPA_OFF] = bf_img.view(fp8)
        critA[:, WPA_OFF:] = blk(0, 0)
        in_maps.append(
            {
                "critA": np.ascontiguousarray(critA),
                "critB": critB,
                "critC": critC,
            }
        )
    return in_maps


def run(inputs, **spmd_kwargs):
    """Build, run on 8 cores, return (output [512,1] f32, BassKernelResults)."""
    nc = _build_program()
    in_maps = _prep_inputs(**inputs)
    res = run_bass_kernel_spmd(nc, in_maps, list(range(N_CORES)), **spmd_kwargs)
    out = np.concatenate(
        [res.results[c]["out"].reshape(BC) for c in range(N_CORES)]
    ).reshape(B, 1).astype(np.float32)
    if HOST_SIG:
        bo = float(np.asarray(inputs["bo"]).reshape(1)[0])
        out = 1.0 / (1.0 + np.exp(-(out / S_T + bo)))
    return out, res


def kernel(**inputs) -> np.ndarray:
    out, _ = run(inputs)
    return out


# revision 27
# speedup vs baseline: 1.0922x; 1.0922x over previous
"""DeepFM forward on Trainium2, 8 NeuronCores, data-parallel over batch.

Reference computes (B=512, n=512, K=4, H=128, n_pairs=130816):
    S  = fm_w @ fm_w.T
    fm = x[:, i1] * x[:, i2] * S[i1, i2]        # [B, n_pairs]
    h2 = relu(relu(x@w1+b1)@w2+b2)
    out = sigmoid(concat([fm, h2]) @ wo + bo)

The fm @ wo[:n_pairs] contraction is the bilinear form
    t1[b] = x[b]^T Wp' x[b]  with  Wp'[i,j] = S[i,j] * Wp[i,j]
where Wp is wo[:n_pairs] scattered into the strictly-upper triangle of a
[n, n] matrix (host-side static relayout). S = fm_w fm_w^T has rank 4, so
    t1[b] = sum_t z_t[b]^T Wp z_t[b],  z_t = x * fm_w[:, t]
and only the 10 upper-triangular 128x128 blocks of Wp are shipped.

All matmul operands are fp8e4 (TRN e4m3, +-240) with power-of-2 scales
chosen so every tensor sits mid-range; the final sigmoid applies the
inverse scale. This halves HBM traffic vs bf16 (the DMA is the dominant
cost) with ~7e-4 relative error (threshold 2e-2).

Everything lives in the [batch, t] free layout so the final fold over t
is one Vector tensor_reduce. Per-core program (BC=64 batch cols):
    z_k[:, b, t] = xT_k * fmw16[k][:, t]        (DVE/Pool, fp8 out)
    VT_j = sum_{k<=j} Wp[k,j]^T @ z_k           (PE fp8 DoubleRow: adjacent
                                                 k-pairs in one instruction)
    Q_j  = VT_j * z_j                           (DVE, bf16)
    t_ps[1, b, t] += ones^T @ Q_j               (PE)
    h1 = relu8(w1^T xT + 16 b1); h2 = relu16(w2^T h1 + 32 b2)  (PE+ACT)
    t_ps[1, b, 0] += (2048 woh)^T @ h2          (PE, strided psum slot)
    t = reduce_t(t_ps); out = sigmoid(t/65536 + bo)            (DVE+ACT)

DMA plan (fixed ~700ns per dma_start on the issuing queue; transfers
share the SDMA rings): sync carries critA (x, fm_w, biases, wp00) then
critB (wp j2/j3 blocks); scalar carries critC (w1, w2, wp j1) in
parallel. PE is HAM-warmed with dummy fp8 matmuls during the DMA wait,
and a few post-output dummy ops keep the clock up into the teardown.
"""

import os
import sys

import numpy as np

for _p in ("/opt/trn_rl_repo", "/root/.axon_site/_ro/trn_rl_repo"):
    if os.path.isdir(_p) and _p not in sys.path:
        sys.path.insert(0, _p)

import ml_dtypes

import concourse.bass as bass
import concourse.tile as tile
from concourse import bacc, mybir
from concourse.bass_utils import run_bass_kernel_spmd

F32 = mybir.dt.float32
BF16 = mybir.dt.bfloat16
FP8 = mybir.dt.float8e4
AF = mybir.ActivationFunctionType
ALU = mybir.AluOpType

N = 512          # n_feat
KFM = 4          # fm embedding dim
H = 128          # mlp hidden
NP = N * (N - 1) // 2
B = 512
N_CORES = 8
BC = B // N_CORES  # 64 batch rows per core
NCH = N // 128     # 4 feature chunks

# fp8 scales (powers of two)
S_FMW = 16.0
S_WP = 256.0
S_W1 = 16.0
S_W2 = 2.0
S_T = S_FMW * S_FMW * S_WP          # 65536 on both t contributions
S_WOH = S_T / (S_W1 * S_W2)         # 2048

# 5 x 512-col dummies end right as the first real operands land; more
# dummies keep the PE busy past that point and delay VT0/h1 (~1us).
N_WARM = int(os.environ.get("DFM_N_WARM", "5"))
WARM_COLS = int(os.environ.get("DFM_WARM_COLS", "512"))
RELU_ENG = os.environ.get("DFM_RELU_ENG", "scalar")
Z3_ENG = os.environ.get("DFM_Z3_ENG", "gpsimd")
HAM_TAIL = int(os.environ.get("DFM_HAM_TAIL", "2"))
B_ENG = os.environ.get("DFM_B_ENG", "sync")      # sync | scalar | gpsimd
DEEP_SLOT = os.environ.get("DFM_DEEP_SLOT", "1") == "1"
PRIME_DMA = os.environ.get("DFM_PRIME_DMA", "0") == "1"
HOST_SIG = os.environ.get("DFM_HOST_SIG", "0") == "1"
DOUBLE_ROW = os.environ.get("DFM_DR", "1") == "1"
# gpsimd cannot read PSUM (walrus rejects it) — Q muls stay on Vector
Q2_ENG = os.environ.get("DFM_Q2_ENG", "vector")

# critA image (fp8 bytes): [ xT (4*64) | f32 pack (19*4) | woh bf16 | ones bf16 | wp(0,0) ]
XT_OFF = 0
F32_OFF = NCH * BC               # 256
F32_COLS = NCH * KFM + 3         # fmw16 | b1*16 | b2*32 | bo  -> 19
BF_OFF = F32_OFF + F32_COLS * 4  # 332
WPA_OFF = BF_OFF + 4             # 336
A_COLS = WPA_OFF + 128           # 464
PK_B1 = NCH * KFM                # 16
PK_B2 = PK_B1 + 1
PK_BO = PK_B2 + 1

# critC: [ w1*16 (4*128) | w2*2 (128) | wp(0,1) | wp(1,1) ]
C_W2 = NCH * H                   # 512
C_WP0 = C_W2 + H                 # 640
C_COLS = C_WP0 + 2 * 128         # 896

# critB: wp blocks (0,2),(1,2),(2,2),(0,3),(1,3),(2,3),(3,3)
B_BLOCKS = [(0, 2), (1, 2), (2, 2), (0, 3), (1, 3), (2, 3), (3, 3)]
B_COLS = len(B_BLOCKS) * 128     # 896

_IU1, _IU2 = np.triu_indices(N, k=1)

_program_cache = None


def _build_program():
    global _program_cache
    if _program_cache is not None:
        return _program_cache

    nc = bacc.Bacc(
        "TRN2", target_bir_lowering=False, debug=False, num_devices=N_CORES
    )
    critA_d = nc.declare_dram_parameter("critA", [128, A_COLS], FP8, isOutput=False)
    critB_d = nc.declare_dram_parameter("critB", [128, B_COLS], FP8, isOutput=False)
    critC_d = nc.declare_dram_parameter("critC", [128, C_COLS], FP8, isOutput=False)
    out_d = nc.declare_dram_parameter("out", [1, BC], F32, isOutput=True)

    relu_eng_is_scalar = RELU_ENG == "scalar"

    with tile.TileContext(nc) as tc:
        with (
            tc.tile_pool(name="const", bufs=1) as cpool,
            tc.tile_pool(name="work", bufs=1) as wpool,
            tc.tile_pool(name="ps_v", bufs=1, space=bass.MemorySpace.PSUM) as vpool,
            tc.tile_pool(name="ps_h", bufs=1, space=bass.MemorySpace.PSUM) as hpool,
            tc.tile_pool(name="ps_t", bufs=1, space=bass.MemorySpace.PSUM) as tpool,
        ):
            # ---- input DMAs: sync gets A then B, scalar gets C ----
            if PRIME_DMA:
                prime_sb = cpool.tile([128, 8], FP8)
                nc.gpsimd.dma_start(prime_sb[:], critB_d[:, 0:8])
            critA_sb = cpool.tile([128, A_COLS], FP8)
            nc.sync.dma_start(critA_sb[:], critA_d[:, :])
            critC_sb = cpool.tile([128, C_COLS], FP8)
            nc.scalar.dma_start(critC_sb[:], critC_d[:, :])
            critB_sb = cpool.tile([128, B_COLS], FP8)
            b_eng = {"sync": nc.sync, "scalar": nc.scalar, "gpsimd": nc.gpsimd}[B_ENG]
            b_eng.dma_start(critB_sb[:], critB_d[:, :])

            # ---- views into critA ----
            f32v = critA_sb[:, F32_OFF:BF_OFF].bitcast(F32)       # [128, 19]
            bf16v = critA_sb[:, BF_OFF:WPA_OFF].bitcast(BF16)     # [128, 2]
            woh_ap = bf16v[:, 0:1]
            ones_ap = bf16v[:, 1:2]
            b1_ap = f32v[:, PK_B1 : PK_B1 + 1]
            b2_ap = f32v[:, PK_B2 : PK_B2 + 1]
            bo_ap = f32v[0:1, PK_BO : PK_BO + 1]

            def xt(k):
                return critA_sb[:, XT_OFF + k * BC : XT_OFF + (k + 1) * BC]

            def fmw(k):
                return f32v[:, k * KFM : (k + 1) * KFM]

            wpA = critA_sb[:, WPA_OFF : WPA_OFF + 128]

            def w1c(k):
                return critC_sb[:, k * H : (k + 1) * H]

            w2_ap = critC_sb[:, C_W2 : C_W2 + H]
            wp_blk = {(0, 0): wpA}
            wp_blk[(0, 1)] = critC_sb[:, C_WP0 : C_WP0 + 128]
            wp_blk[(1, 1)] = critC_sb[:, C_WP0 + 128 : C_WP0 + 256]
            for i, kj in enumerate(B_BLOCKS):
                wp_blk[kj] = critB_sb[:, i * 128 : (i + 1) * 128]

            # ---- warm tiles (GpSimd memsets; Vector stays free) ----
            dum_lhs = cpool.tile([128, 128], FP8)
            nc.gpsimd.memset(dum_lhs[:], 0.0)
            dum_rhs = cpool.tile([128, WARM_COLS], FP8)
            nc.gpsimd.memset(dum_rhs[:], 0.0)
            warm_in = cpool.tile([1, 1], F32)
            nc.gpsimd.memset(warm_in[:], 0.0)

            warm_cols = WARM_COLS if DEEP_SLOT else BC
            warm_ps = hpool.tile(
                [128, warm_cols], F32, tag="warm" if DEEP_SLOT else "h1"
            )
            for d in range(N_WARM):
                nc.tensor.matmul(
                    warm_ps[:], dum_lhs[:], dum_rhs[:, :warm_cols],
                    start=True, stop=True,
                )
            if not HOST_SIG:
                warm_out = cpool.tile([1, 1], F32)
                nc.scalar.activation(warm_out[:], warm_in[:], AF.Sigmoid, bias=0.0)

            # ---- z_k [128, BC, KFM] fp8: x column-scaled by fm_w (rank-4) ----
            z_all = wpool.tile([128, NCH, BC, KFM], FP8, name="z_all", tag="z")

            def z(k):
                return z_all[:, k]

            for k in range(NCH):
                eng = nc.vector if (k < 3 or Z3_ENG == "vector") else nc.gpsimd
                eng.tensor_mul(
                    z(k),
                    xt(k)[:, :, None].broadcast_to([128, BC, KFM]),
                    fmw(k)[:, None, :].broadcast_to([128, BC, KFM]),
                )

            # ---- psum tiles ----
            vt = [
                vpool.tile([128, BC, KFM], F32, name=f"vt{j}", tag=f"v{j}")
                for j in range(NCH)
            ]
            h1_ps = hpool.tile([H, BC], F32, tag="h1")
            h2_ps = hpool.tile([H, BC], F32, tag="h2")
            t_ps = tpool.tile([1, BC, KFM], F32, tag="t")
            deep_out = (
                t_ps[:, :, 0:1] if DEEP_SLOT else tpool.tile([1, BC], F32, tag="deep")[:]
            )

            q_all = wpool.tile([128, NCH, BC, KFM], BF16, name="q_all", tag="q")

            def q(j):
                return q_all[:, j]

            h1_sb = wpool.tile([H, BC], FP8, name="h1_sb")
            h2_sb = wpool.tile([H, BC], BF16, name="h2_sb")

            def relu(dst, src, bias_ap):
                if relu_eng_is_scalar:
                    nc.scalar.activation(dst, src, AF.Relu, bias=bias_ap)
                else:
                    nc.vector.tensor_scalar(
                        dst, src, bias_ap, 0.0, op0=ALU.add, op1=ALU.max
                    )

            # ---- PE stream interleaved with DVE/ACT consumers ----
            # VT j0 (needs critA only)
            nc.tensor.matmul(vt[0][:], wpA, z(0), start=True, stop=True)
            # h1 (needs critC)
            for k in range(NCH):
                nc.tensor.matmul(
                    h1_ps[:], w1c(k), xt(k), start=(k == 0), stop=(k == NCH - 1)
                )
            relu(h1_sb[:], h1_ps[:], b1_ap)
            def wp_pair(base_ap, off):
                return base_ap[:, off : off + 256].rearrange(
                    "p (two f) -> p two f", two=2
                )

            def vt_dr(j, k, pair_ap, start, stop):
                nc.tensor.matmul(
                    vt[j][:], pair_ap, z_all[:, k : k + 2],
                    start=start, stop=stop,
                    perf_mode=mybir.MatmulPerfMode.DoubleRow,
                    skip_group_check=True,
                )

            # VT j1
            if DOUBLE_ROW:
                vt_dr(1, 0, wp_pair(critC_sb, C_WP0), True, True)
            else:
                nc.tensor.matmul(
                    vt[1][:], wp_blk[(0, 1)], z(0), start=True, stop=False
                )
                nc.tensor.matmul(
                    vt[1][:], wp_blk[(1, 1)], z(1), start=False, stop=True
                )
            # Q0
            nc.vector.tensor_mul(q(0), vt[0][:], z(0))
            # h2
            nc.tensor.matmul(h2_ps[:], w2_ap, h1_sb[:], start=True, stop=True)
            relu(h2_sb[:], h2_ps[:], b2_ap)
            # VT j2
            if DOUBLE_ROW:
                vt_dr(2, 0, wp_pair(critB_sb, 0), True, False)
                nc.tensor.matmul(
                    vt[2][:], wp_blk[(2, 2)], z(2),
                    start=False, stop=True, skip_group_check=True,
                )
            else:
                for i, k in enumerate(range(3)):
                    nc.tensor.matmul(
                        vt[2][:], wp_blk[(k, 2)], z(k),
                        start=(i == 0), stop=(i == 2), skip_group_check=True,
                    )
            # Q1
            nc.vector.tensor_mul(q(1), vt[1][:], z(1))
            # t accumulation group start + deep
            nc.tensor.matmul(
                t_ps[:], ones_ap, q(0), start=True, stop=False,
                skip_group_check=True,
            )
            nc.tensor.matmul(
                deep_out, woh_ap, h2_sb[:],
                start=not DEEP_SLOT, stop=not DEEP_SLOT,
                skip_group_check=True,
            )
            # VT j3
            if DOUBLE_ROW:
                vt_dr(3, 0, wp_pair(critB_sb, 384), True, False)
                vt_dr(3, 2, wp_pair(critB_sb, 640), False, True)
            else:
                for i, k in enumerate(range(4)):
                    nc.tensor.matmul(
                        vt[3][:], wp_blk[(k, 3)], z(k),
                        start=(i == 0), stop=(i == 3), skip_group_check=True,
                    )
            # Q2 off the Vector tail (GpSimd is idle by now), t1, Q3, t2, t3
            q2_eng = nc.gpsimd if Q2_ENG == "gpsimd" else nc.vector
            q2_eng.tensor_mul(q(2), vt[2][:], z(2))
            nc.tensor.matmul(
                t_ps[:], ones_ap, q(1), start=False, stop=False,
                skip_group_check=True,
            )
            nc.vector.tensor_mul(q(3), vt[3][:], z(3))
            nc.tensor.matmul(
                t_ps[:], ones_ap, q(2), start=False, stop=False,
                skip_group_check=True,
            )
            nc.tensor.matmul(
                t_ps[:], ones_ap, q(3), start=False, stop=True,
                skip_group_check=True,
            )

            # ---- fold over t (deep already in slot 0), sigmoid, store ----
            tlog_sb = wpool.tile([1, BC], F32, name="tlog")
            nc.vector.tensor_reduce(
                tlog_sb[:], t_ps[:], axis=mybir.AxisListType.X, op=ALU.add
            )
            if not DEEP_SLOT:
                tlog2 = wpool.tile([1, BC], F32, name="tlog2")
                nc.vector.tensor_add(tlog2[:], tlog_sb[:], deep_out)
                tlog_sb = tlog2
            if HOST_SIG:
                nc.sync.dma_start(out_d[:, :], tlog_sb[:])
            else:
                out_sb = wpool.tile([1, BC], F32, name="out_sb")
                nc.scalar.activation(
                    out_sb[:], tlog_sb[:], AF.Sigmoid, bias=bo_ap, scale=1.0 / S_T
                )
                nc.sync.dma_start(out_d[:, :], out_sb[:])

            # ---- keep HAM clock high into the teardown sweep ----
            for _ in range(HAM_TAIL):
                nc.tensor.matmul(
                    warm_ps[:], dum_lhs[:], dum_rhs[:, :warm_cols],
                    start=True, stop=True,
                )

    nc.compile()
    _program_cache = nc
    return nc


def _q8(a, scale):
    return np.clip(
        np.asarray(a, np.float32) * scale, -240.0, 240.0
    ).astype(ml_dtypes.float8_e4m3fn)


def _chunk_pack(a, cols):
    """[512, cols] row-major -> [128, 4, cols] -> [128, 4*cols]."""
    return np.ascontiguousarray(
        a.reshape(NCH, 128, cols).transpose(1, 0, 2).reshape(128, NCH * cols)
    )


def _prep_inputs(x, fm_w, w1, b1, w2, b2, wo, bo):
    x = np.asarray(x, dtype=np.float32)
    fm_w = np.asarray(fm_w, dtype=np.float32)
    w1 = np.asarray(w1, dtype=np.float32)
    w2 = np.asarray(w2, dtype=np.float32)
    wo = np.asarray(wo, dtype=np.float32).reshape(NP + H)
    b1 = np.asarray(b1, dtype=np.float32).reshape(H)
    b2 = np.asarray(b2, dtype=np.float32).reshape(H)
    bo = np.asarray(bo, dtype=np.float32).reshape(1)

    fp8 = ml_dtypes.float8_e4m3fn
    bf = ml_dtypes.bfloat16

    # Pair weights scattered into the strictly-upper triangle, fp8-scaled.
    wp = np.zeros((N, N), dtype=np.float32)
    wp[_IU1, _IU2] = wo[:NP]
    wp_q = _q8(wp, S_WP)

    def blk(k, j):
        return wp_q[128 * k : 128 * (k + 1), 128 * j : 128 * (j + 1)]

    critB = np.empty((128, B_COLS), dtype=fp8)
    for i, (k, j) in enumerate(B_BLOCKS):
        critB[:, i * 128 : (i + 1) * 128] = blk(k, j)
    critB = np.ascontiguousarray(critB)

    critC = np.empty((128, C_COLS), dtype=fp8)
    critC[:, :C_W2] = _chunk_pack(_q8(w1, S_W1), H)
    critC[:, C_W2:C_WP0] = _q8(w2, S_W2)
    critC[:, C_WP0 : C_WP0 + 128] = blk(0, 1)
    critC[:, C_WP0 + 128 :] = blk(1, 1)
    critC = np.ascontiguousarray(critC)

    f32_img = np.zeros((128, F32_COLS), dtype=np.float32)
    f32_img[:, :PK_B1] = _chunk_pack(fm_w * S_FMW, KFM)
    f32_img[:, PK_B1] = b1 * S_W1
    f32_img[:, PK_B2] = b2 * S_W1 * S_W2
    f32_img[:, PK_BO] = bo[0]
    bf_img = np.zeros((128, 2), dtype=bf)
    bf_img[:, 0] = (wo[NP:] * S_WOH).astype(bf)
    bf_img[:, 1] = bf(1.0)

    xT = np.ascontiguousarray(x.T)                       # [N, B] f32

    in_maps = []
    for c in range(N_CORES):
        critA = np.empty((128, A_COLS), dtype=fp8)
        critA[:, XT_OFF:F32_OFF] = _chunk_pack(
            _q8(xT[:, c * BC : (c + 1) * BC], 1.0), BC
        )
        critA[:, F32_OFF:BF_OFF] = f32_img.view(fp8)
        critA[:, BF_OFF:WPA_OFF] = bf_img.view(fp8)
        critA[:, WPA_OFF:] = blk(0, 0)
        in_maps.append(
            {
                "critA": np.ascontiguousarray(critA),
                "critB": critB,
                "critC": critC,
            }
        )
    return in_maps


def run(inputs, **spmd_kwargs):
    """Build, run on 8 cores, return (output [512,1] f32, BassKernelResults)."""
    nc = _build_program()
    in_maps = _prep_inputs(**inputs)
    res = run_bass_kernel_spmd(nc, in_maps, list(range(N_CORES)), **spmd_kwargs)
    out = np.concatenate(
        [res.results[c]["out"].reshape(BC) for c in range(N_CORES)]
    ).reshape(B, 1).astype(np.float32)
    if HOST_SIG:
        bo = float(np.asarray(inputs["bo"]).reshape(1)[0])
        out = 1.0 / (1.0 + np.exp(-(out / S_T + bo)))
    return out, res


def kernel(**inputs) -> np.ndarray:
    out, _ = run(inputs)
    return out
